# revision 4
# baseline (speedup 1.0000x reference)
"""BitLinear (ternary weight quant + per-token int8 activation quant + GEMM)
Trainium2 Bass/Tile kernel, 8-core SPMD.  v8.

Sharding: tokens (B*S = 8192) split 8 ways; quantized weight (bf16)
replicated per core via a one-time AllGather of per-core 512-col slices.
Host ships the weight slice pre-transposed [i, o] (untimed, once).

Dataflow insight (measured): concurrent DMA slows the PE matmul stream by
~1us per MB (SBUF write-port contention), so total DMA bytes are the
second roofline next to the 2048 N=512 matmuls.  x is therefore read
ONCE per rep in its native [t, i] layout: amax, quantize and bf16-cast
all use per-partition token scales there (no broadcast machinery), and
the transpose to [i, t] runs on the PE as identity matmuls at 1 cyc/row
(bf16), +14us of PE time instead of +16MB of DMA.

Pipeline: token halves; xq half1 is double-buffered so the next rep's
phase B overlaps GEMM(half1).  Variant 'ag': half-outer GEMM, wq streamed
per half in snake order (15 contiguous 4MB loads).  Variant 'so':
slice-outer GEMM, wq streamed once (8 loads) but phase B overlap is
limited to the last slice.

Math notes (exactness):
  - a_q in [-127,127] and w_q in {-1,0,1} are exact in bf16; the PE
    accumulates fp32 integer partial sums < 2^24, so the GEMM is exact.
  - round-to-nearest-even via the fp32 magic-number trick (+1.5*2**23).
  - clip(round(w/s),-1,1) == sign(round(w/s)) because |w/s| <= 2, so the
    ACT Sign function performs unshift+clip+cast in one op.
"""

import numpy as np

B, S, D = 2, 4096, 4096
NCORES = 8
T = B * S                  # 8192 tokens
TSH = T // NCORES          # 1024 tokens per core
WSL = D // NCORES          # 512 weight rows (output cols) per core
P = 128
MAGIC = 1.5 * 2**23        # 12582912.0; forces RNE-to-integer in fp32
EPS = 1e-8
QMAX = 127.0
NELEM = float(D * D)

NT = TSH // P              # 8 token tiles
NI = D // P                # 32 contraction blocks
NS = NCORES                # 8 output slices of 512
OSL = D // NS              # 512 output cols per slice
TH = TSH // 2              # 512 tokens per half
NTH = NT // 2              # 4 token tiles per half
DH = D // 2                # x row tiles processed in two 2048-col chunks

_CACHE: dict = {}


def _build(reps=1, variant='ag'):
    import concourse.bass as bass
    import concourse.mybir as mybir
    import concourse.tile as tile
    from concourse import bacc
    from concourse.masks import make_identity

    f32 = mybir.dt.float32
    bf16 = mybir.dt.bfloat16
    X = mybir.AxisListType.X

    nc = bacc.Bacc(
        "TRN2", target_bir_lowering=False, debug=False, num_devices=NCORES
    )

    tm = variant.startswith('tm')
    gemm_order = 'slice' if 'so' in variant else 'half'
    if tm:
        # Timing-only build: identical instruction stream, but tensors live
        # in Internal DRAM (no per-call H2D/D2H); a [1,1] dummy
        # ExternalOutput keeps the PJRT call shape valid.
        xs = nc.dram_tensor("xs", [TSH, D], f32, kind="Internal").ap()
        wt = nc.dram_tensor("wt", [D, WSL], f32, kind="Internal").ap()
        y = nc.dram_tensor("y", [TSH, D], f32, kind="Internal").ap()
        ydum = nc.dram_tensor("ydum", [1, 1], f32, kind="ExternalOutput").ap()
    else:
        xs = nc.dram_tensor("xs", [TSH, D], f32, kind="ExternalInput").ap()
        wt = nc.dram_tensor("wt", [D, WSL], f32, kind="ExternalInput").ap()
        y = nc.dram_tensor("y", [TSH, D], f32, kind="ExternalOutput").ap()

    with tile.TileContext(nc) as tc:
        with (
            tc.tile_pool(name="stage", bufs=2) as stage,      # [P, DH] f32
            tc.tile_pool(name="xb", bufs=2) as xb_pool,       # [P, DH] bf16
            tc.tile_pool(name="stg2", bufs=2) as stg2,        # [P, WSL] f32
            tc.tile_pool(name="wqb", bufs=1) as wqb_pool,     # [P, OSL] bf16
            tc.tile_pool(name="xq0", bufs=1) as xq0_pool,     # [P, NI, TH] bf16
            tc.tile_pool(name="xq1", bufs=2) as xq1_pool,     # [P, NI, TH] bf16
            tc.tile_pool(name="wqt", bufs=2) as wqt_pool,     # [P, NI, OSL] bf16
            tc.tile_pool(name="ysb", bufs=2) as ysb_pool,     # [P, OSL] f32
            tc.tile_pool(name="small", bufs=1) as small,
            tc.tile_pool(name="sm2", bufs=2) as small2,
            tc.tile_pool(name="py", bufs=4, space="PSUM") as psum_y,
            tc.tile_pool(name="pt", bufs=2, space="PSUM") as psum_t,
            tc.tile_pool(name="dram", bufs=1, space="DRAM") as dram,
        ):
            negm = small.tile([P, 1], f32, tag="negm")
            nc.vector.memset(negm, -MAGIC)
            ident_b = small.tile([P, P], bf16, tag="ident_b")
            make_identity(nc, ident_b)

            # ---- Phase A: partial sum of |wt|, AllReduce -> w_scale ----
            def phase_a():
                partials = small.tile([P, NI], f32, tag="partials")
                for ib in range(NI):
                    st = stg2.tile([P, WSL], f32, tag="stw")
                    nc.sync.dma_start(st, wt[ib * P:(ib + 1) * P, :])
                    nc.vector.tensor_reduce(
                        partials[:, ib:ib + 1], st, axis=X,
                        op=mybir.AluOpType.add, apply_absolute_value=True,
                    )
                pcol = small.tile([P, 1], f32, tag="pcol")
                nc.vector.reduce_sum(pcol, partials, axis=X)

                bounce_in = dram.tile([P, 1], f32, tag="cc_in")
                bounce_out = dram.tile([P, 1], f32, tag="cc_out")
                nc.sync.dma_start(bounce_in, pcol)
                nc.gpsimd.collective_compute(
                    "AllReduce",
                    mybir.AluOpType.add,
                    replica_groups=[list(range(NCORES))],
                    ins=[bounce_in.opt()],
                    outs=[bounce_out.opt()],
                )
                srow = small.tile([1, P], f32, tag="srowa")
                nc.sync.dma_start(srow, bounce_out.rearrange("p one -> one p"))
                stot = small.tile([1, 1], f32, tag="stot")
                nc.vector.reduce_sum(stot, srow, axis=X)
                # w_scale = mean + EPS ; also 1/w_scale and w_scale/127
                ws = small.tile([1, 1], f32, tag="ws")
                nc.vector.tensor_scalar(
                    ws, stot, 1.0 / NELEM, EPS,
                    op0=mybir.AluOpType.mult, op1=mybir.AluOpType.add,
                )
                wr = small.tile([1, 1], f32, tag="wr")
                nc.vector.reciprocal(wr, ws)
                w127 = small.tile([1, 1], f32, tag="w127")
                nc.vector.tensor_scalar_mul(w127, ws, 1.0 / QMAX)
                wr_col = small.tile([P, 1], f32, tag="wr_col")
                nc.gpsimd.partition_broadcast(wr_col, wr)
                w127_col = small.tile([P, 1], f32, tag="w127_col")
                nc.gpsimd.partition_broadcast(w127_col, w127)
                return wr_col, w127_col

            wr_col, w127_col = phase_a()

            def phase_a2(wrc):
                # quantize own W slice in [i, o] layout, AllGather
                ag_in = dram.tile([NI, P, OSL], bf16, tag="ag_in")
                for ib in range(NI):
                    st = stg2.tile([P, WSL], f32, tag="stw")
                    nc.sync.dma_start(st, wt[ib * P:(ib + 1) * P, :])
                    nc.vector.tensor_scalar(
                        st, st, wrc, MAGIC,
                        op0=mybir.AluOpType.mult, op1=mybir.AluOpType.add,
                    )
                    # sign(v - MAGIC) == clip(round(w/s), -1, 1); bf16 out
                    wqb = wqb_pool.tile([P, OSL], bf16, tag="wqb")
                    nc.scalar.activation(
                        wqb, st,
                        mybir.ActivationFunctionType.Sign, bias=negm,
                    )
                    nc.sync.dma_start(ag_in[ib], wqb)
                ag_out = dram.tile(
                    [NCORES, NI, P, OSL], bf16, tag="ag_out",
                    addr_space="Shared",
                )
                nc.gpsimd.collective_compute(
                    "AllGather",
                    mybir.AluOpType.bypass,
                    replica_groups=[list(range(NCORES))],
                    ins=[ag_in.opt()],
                    outs=[ag_out.opt()],
                )
                return ag_out

            ag_out = phase_a2(wr_col)

            # ---- Body (repeated for benchmarking) ----
            def body():
                # Phase B: per token tile: amax -> scales, in-place quant
                # (+MAGIC shift), bf16 cast, PE transpose into xq halves.
                comb = small2.tile([P, NT], f32, tag="comb")
                xqh0 = xq0_pool.tile([P, NI, TH], bf16, tag="xq0")
                xqh1 = xq1_pool.tile([P, NI, TH], bf16, tag="xq1")
                xqh = [xqh0, xqh1]
                for t in range(NT):
                    h, tl = divmod(t, NTH)
                    sth = []
                    am2 = small2.tile([P, 2], f32, tag="am2")
                    for hh in range(2):
                        st = stage.tile([P, DH], f32, tag="stage")
                        sth.append(st)
                        nc.sync.dma_start(
                            st, xs[t * P:(t + 1) * P, hh * DH:(hh + 1) * DH]
                        )
                        nc.vector.tensor_reduce(
                            am2[:, hh:hh + 1], st, axis=X,
                            op=mybir.AluOpType.max, apply_absolute_value=True,
                        )
                    a_scale = small2.tile([P, 1], f32, tag="a_scale")
                    nc.vector.tensor_reduce(
                        a_scale, am2, axis=X, op=mybir.AluOpType.max,
                    )
                    nc.vector.tensor_scalar_add(a_scale, a_scale, EPS)
                    arec = small2.tile([P, 1], f32, tag="arec")
                    nc.vector.reciprocal(arec, a_scale)
                    r127 = small2.tile([P, 1], f32, tag="r127")
                    nc.vector.tensor_scalar_mul(r127, arec, QMAX)
                    nc.vector.tensor_scalar(
                        comb[:, t:t + 1], a_scale, w127_col, None,
                        op0=mybir.AluOpType.mult,
                    )
                    for hh in range(2):
                        st = sth[hh]
                        # in-place: st <- st * r127 + MAGIC (RNE + shift)
                        nc.vector.tensor_scalar(
                            st, st, r127, MAGIC,
                            op0=mybir.AluOpType.mult, op1=mybir.AluOpType.add,
                        )
                        # unshift to bf16: transposes at 1 cyc/row + FWL
                        xb = xb_pool.tile([P, DH], bf16, tag="xb")
                        nc.vector.tensor_scalar_sub(xb, st, MAGIC)
                        for g in range(2):
                            ps = psum_t.tile([P, 1024], f32, tag="pt")
                            for bq in range(8):
                                jb = g * 8 + bq
                                nc.tensor.matmul(
                                    ps[:, bq * P:(bq + 1) * P],
                                    lhsT=xb[:, jb * P:(jb + 1) * P],
                                    rhs=ident_b,
                                    start=True, stop=True,
                                )
                            ib0 = hh * (NI // 2) + g * 8
                            nc.scalar.activation(
                                xqh[h][:, ib0:ib0 + 8, tl * P:(tl + 1) * P],
                                ps.rearrange("p (a b) -> p a b", b=P),
                                mybir.ActivationFunctionType.Copy,
                            )

                # Phase C/D: GEMM.
                def gemm_st(s, t, wqT):
                    py = psum_y.tile([P, OSL], f32, tag="py")
                    h, tl = divmod(t, NTH)
                    for i in range(NI):
                        nc.tensor.matmul(
                            py,
                            lhsT=xqh[h][:, i, tl * P:(tl + 1) * P],
                            rhs=wqT[:, i, :],
                            start=(i == 0),
                            stop=(i == NI - 1),
                        )
                    yt = ysb_pool.tile([P, OSL], f32, tag="ysb")
                    nc.scalar.mul(yt, py, comb[:, t:t + 1])
                    nc.sync.dma_start(
                        y[t * P:(t + 1) * P, s * OSL:(s + 1) * OSL], yt
                    )

                def load_wq(s):
                    wqT = wqt_pool.tile([P, NI, OSL], bf16, tag="wqt")
                    nc.sync.dma_start(
                        wqT, ag_out[s].rearrange("b p o -> p b o")
                    )
                    return wqT

                if gemm_order == 'half':
                    # half-outer, snake over slices; s=7 wq tile reused at
                    # the half boundary (15 contiguous 4MB loads per rep)
                    wq_last = None
                    for h in range(2):
                        srange = range(NS) if h == 0 else range(NS - 1, -1, -1)
                        for s in srange:
                            if h == 1 and s == NS - 1 and wq_last is not None:
                                wqT = wq_last
                            else:
                                wqT = load_wq(s)
                            for tl in range(NTH):
                                gemm_st(s, h * NTH + tl, wqT)
                            wq_last = wqT
                else:
                    # slice-outer: wq streamed once (8 loads per rep)
                    for s in range(NS):
                        wqT = load_wq(s)
                        for t in range(NT):
                            gemm_st(s, t, wqT)

            if reps == 1:
                body()
            elif variant == 'unroll':
                for _ in range(reps):
                    body()
            else:
                with tc.For_i(0, reps, 1):
                    body()
            if tm:
                nc.sync.dma_start(ydum, negm[0:1, 0:1])

    nc.compile()
    return nc


def _get_nc(reps=1, variant='ag'):
    key = f"nc{reps}-{variant}"
    if key not in _CACHE:
        _CACHE[key] = _build(reps, variant)
    return _CACHE[key]


def make_in_maps(x, weight):
    x = np.ascontiguousarray(np.asarray(x, dtype=np.float32))
    weight = np.ascontiguousarray(np.asarray(weight, dtype=np.float32))
    xf = x.reshape(T, D)
    wT = np.ascontiguousarray(weight.T)       # [in, out]
    in_maps = []
    for c in range(NCORES):
        in_maps.append({
            "xs": xf[c * TSH:(c + 1) * TSH],
            "wt": np.ascontiguousarray(wT[:, c * WSL:(c + 1) * WSL]),
        })
    return in_maps


def run(x, weight, trace=False, variant="ag", reps=1):
    from concourse.bass_utils import run_bass_kernel_spmd

    nc = _get_nc(reps, variant)
    in_maps = make_in_maps(x, weight)
    res = run_bass_kernel_spmd(
        nc, in_maps, core_ids=list(range(NCORES)), trace=trace
    )
    yf = np.concatenate([res.results[c]["y"] for c in range(NCORES)], axis=0)
    return yf.reshape(B, S, D), res


def kernel(x, weight):
    out, _ = run(x, weight, trace=False)
    return out


# revision 5
# speedup vs baseline: 1.0600x; 1.0600x over previous
"""BitLinear (ternary weight quant + per-token int8 activation quant + GEMM)
Trainium2 Bass/Tile kernel, 8-core SPMD.  v8.

Sharding: tokens (B*S = 8192) split 8 ways; quantized weight (bf16)
replicated per core via a one-time AllGather of per-core 512-col slices.
Host ships the weight slice pre-transposed [i, o] (untimed, once).

Dataflow insight (measured): concurrent DMA slows the PE matmul stream by
~1us per MB (SBUF write-port contention), so total DMA bytes are the
second roofline next to the 2048 N=512 matmuls.  x is therefore read
ONCE per rep in its native [t, i] layout: amax, quantize and bf16-cast
all use per-partition token scales there (no broadcast machinery), and
the transpose to [i, t] runs on the PE as identity matmuls at 1 cyc/row
(bf16), +14us of PE time instead of +16MB of DMA.

Pipeline: token halves; xq half1 is double-buffered so the next rep's
phase B overlaps GEMM(half1).  Variant 'ag': half-outer GEMM, wq streamed
per half in snake order (15 contiguous 4MB loads).  Variant 'so':
slice-outer GEMM, wq streamed once (8 loads) but phase B overlap is
limited to the last slice.

Math notes (exactness):
  - a_q in [-127,127] and w_q in {-1,0,1} are exact in bf16; the PE
    accumulates fp32 integer partial sums < 2^24, so the GEMM is exact.
  - round-to-nearest-even via the fp32 magic-number trick (+1.5*2**23).
  - clip(round(w/s),-1,1) == sign(round(w/s)) because |w/s| <= 2, so the
    ACT Sign function performs unshift+clip+cast in one op.
"""

import numpy as np

B, S, D = 2, 4096, 4096
NCORES = 8
T = B * S                  # 8192 tokens
TSH = T // NCORES          # 1024 tokens per core
WSL = D // NCORES          # 512 weight rows (output cols) per core
P = 128
MAGIC = 1.5 * 2**23        # 12582912.0; forces RNE-to-integer in fp32
EPS = 1e-8
QMAX = 127.0
NELEM = float(D * D)

NT = TSH // P              # 8 token tiles
NI = D // P                # 32 contraction blocks
NS = NCORES                # 8 output slices of 512
OSL = D // NS              # 512 output cols per slice
TH = TSH // 2              # 512 tokens per half
NTH = NT // 2              # 4 token tiles per half
DH = D // 2                # x row tiles processed in two 2048-col chunks

_CACHE: dict = {}


def _build(reps=1, variant='ag'):
    import concourse.bass as bass
    import concourse.mybir as mybir
    import concourse.tile as tile
    from concourse import bacc
    from concourse.masks import make_identity

    f32 = mybir.dt.float32
    bf16 = mybir.dt.bfloat16
    X = mybir.AxisListType.X

    nc = bacc.Bacc(
        "TRN2", target_bir_lowering=False, debug=False, num_devices=NCORES
    )

    tm = variant.startswith('tm')
    gemm_order = 'slice' if 'so' in variant else 'half'
    if tm:
        # Timing-only build: identical instruction stream, but tensors live
        # in Internal DRAM (no per-call H2D/D2H); a [1,1] dummy
        # ExternalOutput keeps the PJRT call shape valid.
        xs = nc.dram_tensor("xs", [TSH, D], f32, kind="Internal").ap()
        wt = nc.dram_tensor("wt", [D, WSL], f32, kind="Internal").ap()
        y = nc.dram_tensor("y", [TSH, D], f32, kind="Internal").ap()
        ydum = nc.dram_tensor("ydum", [1, 1], f32, kind="ExternalOutput").ap()
    else:
        xs = nc.dram_tensor("xs", [TSH, D], f32, kind="ExternalInput").ap()
        wt = nc.dram_tensor("wt", [D, WSL], f32, kind="ExternalInput").ap()
        y = nc.dram_tensor("y", [TSH, D], f32, kind="ExternalOutput").ap()

    with tile.TileContext(nc) as tc:
        with (
            tc.tile_pool(name="stage", bufs=2) as stage,      # [P, DH] f32
            tc.tile_pool(name="xb", bufs=2) as xb_pool,       # [P, DH] bf16
            tc.tile_pool(name="stg2", bufs=2) as stg2,        # [P, WSL] f32
            tc.tile_pool(name="wqb", bufs=1) as wqb_pool,     # [P, OSL] bf16
            tc.tile_pool(name="xq0", bufs=1) as xq0_pool,     # [P, NI, TH] bf16
            tc.tile_pool(name="xq1", bufs=2) as xq1_pool,     # [P, NI, TH] bf16
            tc.tile_pool(name="wqt", bufs=2) as wqt_pool,     # [P, NI, OSL] bf16
            tc.tile_pool(name="ysb", bufs=2) as ysb_pool,     # [P, OSL] f32
            tc.tile_pool(name="small", bufs=1) as small,
            tc.tile_pool(name="sm2", bufs=2) as small2,
            tc.tile_pool(name="py", bufs=4, space="PSUM") as psum_y,
            tc.tile_pool(name="pt", bufs=2, space="PSUM") as psum_t,
            tc.tile_pool(name="dram", bufs=1, space="DRAM") as dram,
        ):
            negm = small.tile([P, 1], f32, tag="negm")
            nc.vector.memset(negm, -MAGIC)
            ident_b = small.tile([P, P], bf16, tag="ident_b")
            make_identity(nc, ident_b)

            # ---- Phase A: partial sum of |wt|, AllReduce -> w_scale ----
            def phase_a():
                partials = small.tile([P, NI], f32, tag="partials")
                for ib in range(NI):
                    st = stg2.tile([P, WSL], f32, tag="stw")
                    nc.sync.dma_start(st, wt[ib * P:(ib + 1) * P, :])
                    nc.vector.tensor_reduce(
                        partials[:, ib:ib + 1], st, axis=X,
                        op=mybir.AluOpType.add, apply_absolute_value=True,
                    )
                pcol = small.tile([P, 1], f32, tag="pcol")
                nc.vector.reduce_sum(pcol, partials, axis=X)

                bounce_in = dram.tile([P, 1], f32, tag="cc_in")
                bounce_out = dram.tile([P, 1], f32, tag="cc_out")
                nc.sync.dma_start(bounce_in, pcol)
                nc.gpsimd.collective_compute(
                    "AllReduce",
                    mybir.AluOpType.add,
                    replica_groups=[list(range(NCORES))],
                    ins=[bounce_in.opt()],
                    outs=[bounce_out.opt()],
                )
                srow = small.tile([1, P], f32, tag="srowa")
                nc.sync.dma_start(srow, bounce_out.rearrange("p one -> one p"))
                stot = small.tile([1, 1], f32, tag="stot")
                nc.vector.reduce_sum(stot, srow, axis=X)
                # w_scale = mean + EPS ; also 1/w_scale and w_scale/127
                ws = small.tile([1, 1], f32, tag="ws")
                nc.vector.tensor_scalar(
                    ws, stot, 1.0 / NELEM, EPS,
                    op0=mybir.AluOpType.mult, op1=mybir.AluOpType.add,
                )
                wr = small.tile([1, 1], f32, tag="wr")
                nc.vector.reciprocal(wr, ws)
                w127 = small.tile([1, 1], f32, tag="w127")
                nc.vector.tensor_scalar_mul(w127, ws, 1.0 / QMAX)
                wr_col = small.tile([P, 1], f32, tag="wr_col")
                nc.gpsimd.partition_broadcast(wr_col, wr)
                w127_col = small.tile([P, 1], f32, tag="w127_col")
                nc.gpsimd.partition_broadcast(w127_col, w127)
                return wr_col, w127_col

            wr_col, w127_col = phase_a()

            def phase_a2(wrc):
                # quantize own W slice in [i, o] layout, AllGather
                ag_in = dram.tile([NI, P, OSL], bf16, tag="ag_in")
                for ib in range(NI):
                    st = stg2.tile([P, WSL], f32, tag="stw")
                    nc.sync.dma_start(st, wt[ib * P:(ib + 1) * P, :])
                    nc.vector.tensor_scalar(
                        st, st, wrc, MAGIC,
                        op0=mybir.AluOpType.mult, op1=mybir.AluOpType.add,
                    )
                    # sign(v - MAGIC) == clip(round(w/s), -1, 1); bf16 out
                    wqb = wqb_pool.tile([P, OSL], bf16, tag="wqb")
                    nc.scalar.activation(
                        wqb, st,
                        mybir.ActivationFunctionType.Sign, bias=negm,
                    )
                    nc.sync.dma_start(ag_in[ib], wqb)
                ag_out = dram.tile(
                    [NCORES, NI, P, OSL], bf16, tag="ag_out",
                    addr_space="Shared",
                )
                nc.gpsimd.collective_compute(
                    "AllGather",
                    mybir.AluOpType.bypass,
                    replica_groups=[list(range(NCORES))],
                    ins=[ag_in.opt()],
                    outs=[ag_out.opt()],
                )
                return ag_out

            ag_out = phase_a2(wr_col)
            # Copy the gathered weights from the Shared-space collective
            # buffer to plain local DRAM once (pre-body, untimed): body
            # streams 63MB/rep from here, and Shared-space reads may carry
            # a routing penalty.
            wq_local = dram.tile(
                [NCORES, NI, P, OSL], bf16, tag="wq_local"
            )
            for s in range(NCORES):
                nc.sync.dma_start(wq_local[s], ag_out[s])
            ag_out = wq_local

            # ---- Body (repeated for benchmarking) ----
            def body():
                # Phase B: per token tile: amax -> scales, in-place quant
                # (+MAGIC shift), bf16 cast, PE transpose into xq halves.
                comb = small2.tile([P, NT], f32, tag="comb")
                xqh0 = xq0_pool.tile([P, NI, TH], bf16, tag="xq0")
                xqh1 = xq1_pool.tile([P, NI, TH], bf16, tag="xq1")
                xqh = [xqh0, xqh1]
                for t in range(NT):
                    h, tl = divmod(t, NTH)
                    sth = []
                    am2 = small2.tile([P, 2], f32, tag="am2")
                    for hh in range(2):
                        st = stage.tile([P, DH], f32, tag="stage")
                        sth.append(st)
                        nc.sync.dma_start(
                            st, xs[t * P:(t + 1) * P, hh * DH:(hh + 1) * DH]
                        )
                        nc.vector.tensor_reduce(
                            am2[:, hh:hh + 1], st, axis=X,
                            op=mybir.AluOpType.max, apply_absolute_value=True,
                        )
                    a_scale = small2.tile([P, 1], f32, tag="a_scale")
                    nc.vector.tensor_reduce(
                        a_scale, am2, axis=X, op=mybir.AluOpType.max,
                    )
                    nc.vector.tensor_scalar_add(a_scale, a_scale, EPS)
                    arec = small2.tile([P, 1], f32, tag="arec")
                    nc.vector.reciprocal(arec, a_scale)
                    r127 = small2.tile([P, 1], f32, tag="r127")
                    nc.vector.tensor_scalar_mul(r127, arec, QMAX)
                    nc.vector.tensor_scalar(
                        comb[:, t:t + 1], a_scale, w127_col, None,
                        op0=mybir.AluOpType.mult,
                    )
                    for hh in range(2):
                        st = sth[hh]
                        # in-place: st <- st * r127 + MAGIC (RNE + shift)
                        nc.vector.tensor_scalar(
                            st, st, r127, MAGIC,
                            op0=mybir.AluOpType.mult, op1=mybir.AluOpType.add,
                        )
                        # unshift to bf16: transposes at 1 cyc/row + FWL
                        xb = xb_pool.tile([P, DH], bf16, tag="xb")
                        nc.vector.tensor_scalar_sub(xb, st, MAGIC)
                        for g in range(2):
                            ps = psum_t.tile([P, 1024], f32, tag="pt")
                            for bq in range(8):
                                jb = g * 8 + bq
                                nc.tensor.matmul(
                                    ps[:, bq * P:(bq + 1) * P],
                                    lhsT=xb[:, jb * P:(jb + 1) * P],
                                    rhs=ident_b,
                                    start=True, stop=True,
                                )
                            ib0 = hh * (NI // 2) + g * 8
                            nc.scalar.activation(
                                xqh[h][:, ib0:ib0 + 8, tl * P:(tl + 1) * P],
                                ps.rearrange("p (a b) -> p a b", b=P),
                                mybir.ActivationFunctionType.Copy,
                            )

                # Phase C/D: GEMM.
                def gemm_st(s, t, wqT):
                    py = psum_y.tile([P, OSL], f32, tag="py")
                    h, tl = divmod(t, NTH)
                    for i in range(NI):
                        nc.tensor.matmul(
                            py,
                            lhsT=xqh[h][:, i, tl * P:(tl + 1) * P],
                            rhs=wqT[:, i, :],
                            start=(i == 0),
                            stop=(i == NI - 1),
                        )
                    yt = ysb_pool.tile([P, OSL], f32, tag="ysb")
                    nc.scalar.mul(yt, py, comb[:, t:t + 1])
                    nc.sync.dma_start(
                        y[t * P:(t + 1) * P, s * OSL:(s + 1) * OSL], yt
                    )

                def load_wq(s):
                    wqT = wqt_pool.tile([P, NI, OSL], bf16, tag="wqt")
                    nc.sync.dma_start(
                        wqT, ag_out[s].rearrange("b p o -> p b o")
                    )
                    return wqT

                if gemm_order == 'half':
                    # half-outer, snake over slices; s=7 wq tile reused at
                    # the half boundary (15 contiguous 4MB loads per rep)
                    wq_last = None
                    for h in range(2):
                        srange = range(NS) if h == 0 else range(NS - 1, -1, -1)
                        for s in srange:
                            if h == 1 and s == NS - 1 and wq_last is not None:
                                wqT = wq_last
                            else:
                                wqT = load_wq(s)
                            for tl in range(NTH):
                                gemm_st(s, h * NTH + tl, wqT)
                            wq_last = wqT
                else:
                    # slice-outer: wq streamed once (8 loads per rep)
                    for s in range(NS):
                        wqT = load_wq(s)
                        for t in range(NT):
                            gemm_st(s, t, wqT)

            if reps == 1:
                body()
            elif variant == 'unroll':
                for _ in range(reps):
                    body()
            else:
                with tc.For_i(0, reps, 1):
                    body()
            if tm:
                nc.sync.dma_start(ydum, negm[0:1, 0:1])

    nc.compile()
    return nc


def _get_nc(reps=1, variant='ag'):
    key = f"nc{reps}-{variant}"
    if key not in _CACHE:
        _CACHE[key] = _build(reps, variant)
    return _CACHE[key]


def make_in_maps(x, weight):
    x = np.ascontiguousarray(np.asarray(x, dtype=np.float32))
    weight = np.ascontiguousarray(np.asarray(weight, dtype=np.float32))
    xf = x.reshape(T, D)
    wT = np.ascontiguousarray(weight.T)       # [in, out]
    in_maps = []
    for c in range(NCORES):
        in_maps.append({
            "xs": xf[c * TSH:(c + 1) * TSH],
            "wt": np.ascontiguousarray(wT[:, c * WSL:(c + 1) * WSL]),
        })
    return in_maps


def run(x, weight, trace=False, variant="ag", reps=1):
    from concourse.bass_utils import run_bass_kernel_spmd

    nc = _get_nc(reps, variant)
    in_maps = make_in_maps(x, weight)
    res = run_bass_kernel_spmd(
        nc, in_maps, core_ids=list(range(NCORES)), trace=trace
    )
    yf = np.concatenate([res.results[c]["y"] for c in range(NCORES)], axis=0)
    return yf.reshape(B, S, D), res


def kernel(x, weight):
    out, _ = run(x, weight, trace=False)
    return out
